# revision 5
# baseline (speedup 1.0000x reference)
"""MoE layer (top-2 of 8 experts, D=1024, H=2048) on 8 trn2 NeuronCores.

Strategy (expert-parallel, per the sharding hint):
  - Router (tiny: [16384,1024]@[1024,8]) runs on host; its output IS the
    sharding decision ("all-to-all tokens by expert assignment").
  - Core e receives the tokens routed to expert e (padded to a uniform
    capacity C=4096, packed per 512-token block in partition-major
    layout) plus expert e's weights, and computes on-device:
        stage 1: hT[h, c] = relu(sum_d w1[d,h] * xT[d,c])
        stage 2: yT[d, c] = sum_h w2[h,d] * hT[h,c]
    bf16 matmuls, f32 PSUM accumulation, bf16 output.
  - Host scatter-adds gate * (y + b2) into the output (f32).

Perf notes (v2, from perfetto trace of v1):
  - All dram<->sbuf layouts are partition-major so every DMA moves
    2-32KB per partition in one descriptor (v1's 1-2KB descriptors made
    the first x block + weight head-load take ~7.6us).
  - Block 0's x and w1[m=0] are split into small lead DMAs so the first
    matmul starts ~1.5us after triggers fire instead of ~7.6us.
  - ~10 warmup matmuls on a memset scratch tile run while the first
    DMAs land: they lift the PE HAM clock-gate (1.2->2.4GHz takes
    ~3.4us of sustained PE activity) so the first real matmuls run at
    full clock.
  - y is written out in bf16 (half the DMA bytes of f32; rel-err cost
    ~1e-3, budget is 2e-2), and the last block streams out per d-tile
    so the final DMA after the last matmul is tiny.
"""

import numpy as np
import ml_dtypes

import concourse.bacc as bacc
import concourse.mybir as mybir
import concourse.tile as tile
from concourse.tile_rust import add_dep_helper
from concourse import bass_utils

B, S, D, E, TOPK = 4, 4096, 1024, 8, 2
H = 2 * D
P = 128
KD = D // P    # 8 k-tiles over D
MH = H // P    # 16 h-tiles over H
ND = D // P    # 8 d-tiles over D
FD = 512       # moving free-dim per matmul / column block
NB = 8         # column blocks per core (C = NB*FD = 4096)
C = NB * FD
N_CORES = 8
N_WARMUP = 6  # junk matmuls to lift the HAM clock gate during head DMA

BF16 = mybir.dt.bfloat16
F32 = mybir.dt.float32

_cache = {}           # (C, with_b1) -> compiled Bacc
last_run_results = None  # BassKernelResults of the most recent device run


def _build(with_b1=False):
    """Build + compile the per-core FFN program.

    Dram layouts are partition-major (host pre-packs) so each DMA is a
    few KB contiguous per partition:
      xb [NB, P, KD, FD]   xb[b,p,k,c] = x[k*128+p, b*512+c]   (8KB/part)
      w1p[P, MH, KD*P]     w1p[p,m,(k j)] = w1[k*128+p, m*128+j]
      w2p[P, ND, MH*P]     w2p[p,d,(m j)] = w2[m*128+p, d*128+j]
      yb [NB, P, ND, FD]   yb[b,p,d,c] = y[d*128+p, b*512+c]   bf16
    """
    nc = bacc.Bacc("TRN2", target_bir_lowering=False, debug=False)
    xb = nc.dram_tensor("xb", [NB, P, KD, FD], BF16, kind="ExternalInput").ap()
    w1p = nc.dram_tensor("w1p", [P, MH, KD * P], BF16, kind="ExternalInput").ap()
    w2p = nc.dram_tensor("w2p", [P, ND, MH * P], BF16, kind="ExternalInput").ap()
    b1t = (
        nc.dram_tensor("b1t", [P, MH], F32, kind="ExternalInput").ap()
        if with_b1 else None
    )
    yb = nc.dram_tensor("yb", [NB, P, ND, FD], BF16, kind="ExternalOutput").ap()

    with tile.TileContext(nc) as tc:
        with (
            tc.tile_pool(name="wpool", bufs=1) as wpool,
            tc.tile_pool(name="xpool", bufs=2) as xpool,
            tc.tile_pool(name="hpool", bufs=3) as hpool,
            tc.tile_pool(name="ypool", bufs=2) as ypool,
            tc.tile_pool(name="ps1", bufs=4, space="PSUM") as ps1,
            tc.tile_pool(name="ps2", bufs=4, space="PSUM") as ps2,
        ):
            # Warmup: junk matmuls on a memset scratch tile (results land
            # in PSUM, never read). The memset runs at vector-queue body
            # start (~7.0us) so the PE chews these while the head DMAs
            # are still in flight, lifting the HAM clock gate (~3.4us of
            # sustained PE busy at 1.2GHz) before the first real matmul.
            junk = wpool.tile([P, 128 + FD], BF16)
            nc.vector.memset(junk[:], 0)
            for _ in range(N_WARMUP):
                ps = ps1.tile([P, FD], F32)
                nc.tensor.matmul(
                    ps[:], junk[:, :128], junk[:, 128:], start=True, stop=True
                )

            # Head loads all on ONE ring (sync), as FAT chunks (4-16KB
            # per-partition lines). Per-DMA-engine packet cost is ~130ns
            # fixed + ~21ns/KB, so 1-2KB lines crawl at ~100GB/s
            # aggregate while 4-16KB lines sustain ~300-430GB/s. Chunks
            # are ordered by PE consumption; trigger instructions cost
            # ~600ns each on the queue, so fewer/fatter also starts the
            # stream sooner. All transfers are HWDGE (SWDGE adds ~10us).
            xb0 = xpool.tile([P, KD, FD], BF16)
            w1_sb = wpool.tile([P, MH, KD * P], BF16)
            nc.sync.dma_start(xb0[:, 0:4, :], xb[0, :, 0:4, :])      # 512KB, 4KB lines
            nc.sync.dma_start(w1_sb[:, 0:2, :], w1p[:, 0:2, :])      # 512KB, 4KB lines
            nc.sync.dma_start(w1_sb[:, 2:4, :], w1p[:, 2:4, :])      # 512KB
            nc.sync.dma_start(w1_sb[:, 4:8, :], w1p[:, 4:8, :])      # 1MB, 8KB lines
            nc.sync.dma_start(xb0[:, 4:8, :], xb[0, :, 4:8, :])      # 512KB
            nc.sync.dma_start(w1_sb[:, 8:12, :], w1p[:, 8:12, :])    # 1MB
            nc.sync.dma_start(w1_sb[:, 12:16, :], w1p[:, 12:16, :])  # 1MB
            if with_b1:
                b1_sb = wpool.tile([P, MH], F32)
                nc.scalar.dma_start(b1_sb[:], b1t[:, :])

            # w2 (4MB) and xb1 are only needed ~30us in: gate them on the
            # m8-11 w1 chunk LANDING (a probe copy that reads the data,
            # not just the trigger) so they cannot starve the w1 stream
            # that stage 1 consumes progressively. xb1 first: it is
            # needed before w2's last chunk.
            probe = wpool.tile([1, 8], BF16)
            w1_probe = nc.vector.tensor_copy(probe[:1, :8], w1_sb[:1, 11, 0:8])
            w2_sb = wpool.tile([P, ND, MH * P], BF16)
            xb1t = xpool.tile([P, KD, FD], BF16)
            gated = [
                nc.sync.dma_start(xb1t[:], xb[1, :, :, :]),
                nc.sync.dma_start(w2_sb[:, 0:4, :], w2p[:, 0:4, :]),
                nc.sync.dma_start(w2_sb[:, 4:8, :], w2p[:, 4:8, :]),
            ]
            for g in gated:
                add_dep_helper(
                    g.ins, w1_probe.ins,
                    reason="yield HBM bandwidth to the w1 stream",
                )

            xbs = {}

            def relu_out(hT, m, ps):
                nc.scalar.activation(
                    hT[:, m, :],
                    ps[:],
                    mybir.ActivationFunctionType.Relu,
                    bias=b1_sb[:, m:m + 1] if with_b1 else 0.0,
                )

            def stage1(blki):
                if blki == 0:
                    xbt = xb0
                elif blki == 1:
                    xbt = xb1t
                else:
                    xbt = xpool.tile([P, KD, FD], BF16)
                    nc.sync.dma_start(xbt[:], xb[blki, :, :, :])
                hT = hpool.tile([P, MH, FD], BF16)
                xbs[blki] = hT
                m0 = 0
                if blki == 0:
                    # Deep split-k head: run m0-7 over k0-3 first (start,
                    # no stop, one PSUM bank each — borrowing ps2's 4
                    # banks, idle until stage2(0), for m4-7), then finish
                    # all 8 groups over k4-7. This gives the PE ~32 MMs
                    # of work that needs only the first 2.5MB of head DMA
                    # (x0 k0-3 + w1 m0-7), riding out the DMA ramp; the
                    # x0 k4-7 chunk has until the k4-7 phase (~8us later)
                    # to land.
                    psA = []
                    for m in range(8):
                        ps = (ps1 if m < 4 else ps2).tile([P, FD], F32)
                        psA.append(ps)
                        for k in range(4):
                            nc.tensor.matmul(
                                ps[:],
                                w1_sb[:, m, k * P:(k + 1) * P],
                                xbt[:, k, :],
                                start=(k == 0),
                                stop=False,
                            )
                    for m in range(8):
                        ps = psA[m]
                        for k in range(4, KD):
                            nc.tensor.matmul(
                                ps[:],
                                w1_sb[:, m, k * P:(k + 1) * P],
                                xbt[:, k, :],
                                start=False,
                                stop=(k == KD - 1),
                            )
                        relu_out(hT, m, ps)
                    m0 = 8
                for m in range(m0, MH):
                    ps = ps1.tile([P, FD], F32)
                    for k in range(KD):
                        nc.tensor.matmul(
                            ps[:],
                            w1_sb[:, m, k * P:(k + 1) * P],
                            xbt[:, k, :],
                            start=(k == 0),
                            stop=(k == KD - 1),
                        )
                    relu_out(hT, m, ps)

            def stage2(blki):
                last = blki == NB - 1
                hT = xbs.pop(blki)
                yt = ypool.tile([P, ND, FD], BF16)
                for d in range(ND):
                    # Final d-tile of the final block: split 384/128 so
                    # the very last copy + DMA after the last matmul
                    # cover only 128 columns (the DMA trigger cost is
                    # fixed at 128 descriptors; the copy and data scale).
                    halves = (
                        [(0, 384), (384, FD)]
                        if last and d == ND - 1 else [(0, FD)]
                    )
                    for c0, c1 in halves:
                        ps = ps2.tile([P, c1 - c0], F32)
                        for m in range(MH):
                            nc.tensor.matmul(
                                ps[:],
                                w2_sb[:, d, m * P:(m + 1) * P],
                                hT[:, m, c0:c1],
                                start=(m == 0),
                                stop=(m == MH - 1),
                            )
                        nc.vector.tensor_copy(yt[:, d, c0:c1], ps[:])
                        if last:  # stream the tail out: final DMA is tiny
                            nc.sync.dma_start(
                                yb[blki, :, d, c0:c1], yt[:, d, c0:c1]
                            )
                if not last:
                    nc.sync.dma_start(yb[blki, :, :, :], yt[:])

            # software-pipelined: stage 1 of block b+1 runs (on PE) between
            # stage 1 and stage 2 of block b, hiding the relu-eviction tail
            stage1(0)
            for b in range(NB):
                if b + 1 < NB:
                    stage1(b + 1)
                stage2(b)

    nc.compile()
    return nc


def _route(x_flat, router_w, router_b):
    """Replicates the reference router in numpy float32."""
    logits = x_flat @ router_w + router_b            # [N, E]
    m = logits.max(axis=-1, keepdims=True)
    p = np.exp(logits - m, dtype=np.float32)
    p /= p.sum(axis=-1, keepdims=True)
    # top-k, ties -> lower index first (matches jax.lax.top_k)
    top_i = np.argsort(-p, axis=-1, kind="stable")[:, :TOPK]
    top_p = np.take_along_axis(p, top_i, axis=-1)
    top_p = top_p / top_p.sum(axis=-1, keepdims=True)
    return top_p, top_i


def kernel(x, router_w, router_b, w1, b1, w2, b2, _trace=False):
    global last_run_results
    x = np.asarray(x, dtype=np.float32)
    router_w = np.asarray(router_w, dtype=np.float32)
    router_b = np.asarray(router_b, dtype=np.float32)
    w1 = np.asarray(w1, dtype=np.float32)
    b1 = np.asarray(b1, dtype=np.float32)
    w2 = np.asarray(w2, dtype=np.float32)
    b2 = np.asarray(b2, dtype=np.float32)

    N = B * S
    x_flat = x.reshape(N, D)
    top_p, top_i = _route(x_flat, router_w, router_b)

    # Tokens per expert (the "all-to-all by expert assignment")
    idx = [np.nonzero((top_i == e).any(axis=-1))[0] for e in range(E)]
    gates = [
        (top_p[idx[e]] * (top_i[idx[e]] == e)).sum(axis=-1) for e in range(E)
    ]
    counts = np.array([len(i) for i in idx])
    # Device capacity C=4096: the perfect-balance point (N*K/8). The few
    # overflow tokens of hotter-than-average experts are handled on the
    # host during the scatter-add (a data-parallel remainder).

    with_b1 = bool(np.any(b1))
    if with_b1 not in _cache:
        _cache[with_b1] = _build(with_b1=with_b1)
    nc = _cache[with_b1]

    in_maps = []
    for e in range(E):
        n_e = min(int(counts[e]), C)
        xTe = np.zeros((D, C), dtype=np.float32)
        xTe[:, :n_e] = x_flat[idx[e][:n_e]].T
        # [D, C] -> [NB, P, KD, FD] with D=(k p), C=(b c)
        xbe = np.ascontiguousarray(
            xTe.reshape(KD, P, NB, FD).transpose(2, 1, 0, 3)
        ).astype(ml_dtypes.bfloat16)
        w1pe = np.ascontiguousarray(
            w1[e].reshape(KD, P, MH, P).transpose(1, 2, 0, 3).reshape(P, MH, KD * P)
        ).astype(ml_dtypes.bfloat16)
        w2pe = np.ascontiguousarray(
            w2[e].reshape(MH, P, ND, P).transpose(1, 2, 0, 3).reshape(P, ND, MH * P)
        ).astype(ml_dtypes.bfloat16)
        im = {"xb": xbe, "w1p": w1pe, "w2p": w2pe}
        if with_b1:
            im["b1t"] = np.ascontiguousarray(b1[e].reshape(MH, P).T)
        in_maps.append(im)

    res = None
    for attempt in range(3):
        try:
            res = bass_utils.run_bass_kernel_spmd(
                nc, in_maps, list(range(N_CORES)), trace=_trace
            )
            break
        except Exception:
            if attempt == 2:
                raise
    last_run_results = res

    out_flat = np.zeros((N, D), dtype=np.float32)
    for e in range(E):
        n_e = min(int(counts[e]), C)
        ybe = np.asarray(res.results[e]["yb"]).astype(np.float32)
        # [NB, P, ND, FD] -> [C, D] with C=(b c), D=(d p)
        y_e = ybe.transpose(0, 3, 2, 1).reshape(C, D)[:n_e]
        out_flat[idx[e][:n_e]] += gates[e][:n_e, None] * (y_e + b2[e])
        if counts[e] > C:  # host handles the overflow tokens
            hi = idx[e][C:]
            h = np.maximum(x_flat[hi] @ w1[e] + b1[e], 0.0)
            y = h @ w2[e] + b2[e]
            out_flat[hi] += gates[e][C:, None] * y
    return out_flat.reshape(B, S, D)



# revision 8
# speedup vs baseline: 1.0039x; 1.0039x over previous
"""MoE layer (top-2 of 8 experts, D=1024, H=2048) on 8 trn2 NeuronCores.

Strategy (expert-parallel, per the sharding hint):
  - Router (tiny: [16384,1024]@[1024,8]) runs on host; its output IS the
    sharding decision ("all-to-all tokens by expert assignment").
  - Core e receives the tokens routed to expert e (padded to a uniform
    capacity C=4096, packed per 512-token block in partition-major
    layout) plus expert e's weights, and computes on-device:
        stage 1: hT[h, c] = relu(sum_d w1[d,h] * xT[d,c])
        stage 2: yT[d, c] = sum_h w2[h,d] * hT[h,c]
    bf16 matmuls, f32 PSUM accumulation, bf16 output.
  - Host scatter-adds gate * (y + b2) into the output (f32).

Perf notes (v2, from perfetto trace of v1):
  - All dram<->sbuf layouts are partition-major so every DMA moves
    2-32KB per partition in one descriptor (v1's 1-2KB descriptors made
    the first x block + weight head-load take ~7.6us).
  - Block 0's x and w1[m=0] are split into small lead DMAs so the first
    matmul starts ~1.5us after triggers fire instead of ~7.6us.
  - ~10 warmup matmuls on a memset scratch tile run while the first
    DMAs land: they lift the PE HAM clock-gate (1.2->2.4GHz takes
    ~3.4us of sustained PE activity) so the first real matmuls run at
    full clock.
  - y is written out in bf16 (half the DMA bytes of f32; rel-err cost
    ~1e-3, budget is 2e-2), and the last block streams out per d-tile
    so the final DMA after the last matmul is tiny.
"""

import numpy as np
import ml_dtypes

import concourse.bacc as bacc
import concourse.mybir as mybir
import concourse.tile as tile
from concourse.tile_rust import add_dep_helper
from concourse import bass_utils

B, S, D, E, TOPK = 4, 4096, 1024, 8, 2
H = 2 * D
P = 128
KD = D // P    # 8 k-tiles over D
MH = H // P    # 16 h-tiles over H
ND = D // P    # 8 d-tiles over D
FD = 512       # moving free-dim per matmul / column block
NB = 8         # column blocks per core (C = NB*FD = 4096)
C = NB * FD
N_CORES = 8
N_WARMUP = 10  # junk matmuls to lift the HAM clock gate during head DMA

BF16 = mybir.dt.bfloat16
F32 = mybir.dt.float32

_cache = {}           # (C, with_b1) -> compiled Bacc
last_run_results = None  # BassKernelResults of the most recent device run


def _build(with_b1=False):
    """Build + compile the per-core FFN program.

    Dram layouts are partition-major (host pre-packs) so each DMA is a
    few KB contiguous per partition:
      xb [NB, P, KD, FD]   xb[b,p,k,c] = x[k*128+p, b*512+c]   (8KB/part)
      w1p[P, MH, KD*P]     w1p[p,m,(k j)] = w1[k*128+p, m*128+j]
      w2p[P, ND, MH*P]     w2p[p,d,(m j)] = w2[m*128+p, d*128+j]
      yb [NB, P, ND, FD]   yb[b,p,d,c] = y[d*128+p, b*512+c]   bf16
    """
    nc = bacc.Bacc("TRN2", target_bir_lowering=False, debug=False)
    xb = nc.dram_tensor("xb", [NB, P, KD, FD], BF16, kind="ExternalInput").ap()
    w1p = nc.dram_tensor("w1p", [P, MH, KD * P], BF16, kind="ExternalInput").ap()
    w2p = nc.dram_tensor("w2p", [P, ND, MH * P], BF16, kind="ExternalInput").ap()
    b1t = (
        nc.dram_tensor("b1t", [P, MH], F32, kind="ExternalInput").ap()
        if with_b1 else None
    )
    yb = nc.dram_tensor("yb", [NB, P, ND, FD], BF16, kind="ExternalOutput").ap()

    with tile.TileContext(nc) as tc:
        with (
            tc.tile_pool(name="wpool", bufs=1) as wpool,
            tc.tile_pool(name="xpool", bufs=2) as xpool,
            tc.tile_pool(name="hpool", bufs=3) as hpool,
            tc.tile_pool(name="ypool", bufs=2) as ypool,
            tc.tile_pool(name="ps1", bufs=4, space="PSUM") as ps1,
            tc.tile_pool(name="ps2", bufs=4, space="PSUM") as ps2,
        ):
            # Warmup: junk matmuls on a memset scratch tile (results land
            # in PSUM, never read). The memset runs at vector-queue body
            # start (~7.0us) so the PE chews these while the head DMAs
            # are still in flight, lifting the HAM clock gate (~3.4us of
            # sustained PE busy at 1.2GHz) before the first real matmul.
            junk = wpool.tile([P, 128 + FD], BF16)
            nc.vector.memset(junk[:], 0)
            for _ in range(N_WARMUP):
                ps = ps1.tile([P, FD], F32)
                nc.tensor.matmul(
                    ps[:], junk[:, :128], junk[:, 128:], start=True, stop=True
                )

            # Head loads all on ONE ring (sync), as FAT chunks (4-16KB
            # per-partition lines). Per-DMA-engine packet cost is ~130ns
            # fixed + ~21ns/KB, so 1-2KB lines crawl at ~100GB/s
            # aggregate while 4-16KB lines sustain ~300-430GB/s. Chunks
            # are ordered by PE consumption; trigger instructions cost
            # ~600ns each on the queue, so fewer/fatter also starts the
            # stream sooner. All transfers are HWDGE (SWDGE adds ~10us).
            xb0 = xpool.tile([P, KD, FD], BF16)
            w1_sb = wpool.tile([P, MH, KD * P], BF16)
            # Leads stay SKINNY: the DMA path crawls (~50-150GB/s) for the
            # first ~3us after the doorbell while engines wake, so the
            # bytes gating the FIRST matmul (x0 k0 + w1 m0 = 384KB) must
            # be minimal. The bulk rides the matured ~400GB/s stream in
            # fat chunks, and fewer triggers (~600ns each, serial on the
            # queue) get the whole 5MB in flight by ~11us.
            nc.sync.dma_start(xb0[:, 0:1, :], xb[0, :, 0:1, :])      # 128KB
            nc.sync.dma_start(w1_sb[:, 0, :], w1p[:, 0, :])          # 256KB
            nc.sync.dma_start(xb0[:, 1:4, :], xb[0, :, 1:4, :])      # 384KB
            nc.sync.dma_start(w1_sb[:, 1, :], w1p[:, 1, :])          # 256KB
            nc.sync.dma_start(w1_sb[:, 2:4, :], w1p[:, 2:4, :])      # 512KB
            nc.sync.dma_start(xb0[:, 4:8, :], xb[0, :, 4:8, :])      # 512KB
            nc.sync.dma_start(w1_sb[:, 4:8, :], w1p[:, 4:8, :])      # 1MB
            nc.sync.dma_start(w1_sb[:, 8:12, :], w1p[:, 8:12, :])    # 1MB
            nc.sync.dma_start(w1_sb[:, 12:16, :], w1p[:, 12:16, :])  # 1MB
            if with_b1:
                b1_sb = wpool.tile([P, MH], F32)
                nc.scalar.dma_start(b1_sb[:], b1t[:, :])

            # w2 (4MB) and xb1 are only needed ~30us in: gate them on the
            # m8-11 w1 chunk LANDING (a probe copy that reads the data,
            # not just the trigger) so they cannot starve the w1 stream
            # that stage 1 consumes progressively. xb1 first: it is
            # needed before w2's last chunk.
            probe = wpool.tile([1, 8], BF16)
            w1_probe = nc.vector.tensor_copy(probe[:1, :8], w1_sb[:1, 11, 0:8])
            w2_sb = wpool.tile([P, ND, MH * P], BF16)
            xb1t = xpool.tile([P, KD, FD], BF16)
            gated = [
                nc.sync.dma_start(xb1t[:], xb[1, :, :, :]),
                nc.sync.dma_start(w2_sb[:, 0:4, :], w2p[:, 0:4, :]),
                nc.sync.dma_start(w2_sb[:, 4:8, :], w2p[:, 4:8, :]),
            ]
            for g in gated:
                add_dep_helper(
                    g.ins, w1_probe.ins,
                    reason="yield HBM bandwidth to the w1 stream",
                )

            xbs = {}

            def relu_out(hT, m, ps):
                nc.scalar.activation(
                    hT[:, m, :],
                    ps[:],
                    mybir.ActivationFunctionType.Relu,
                    bias=b1_sb[:, m:m + 1] if with_b1 else 0.0,
                )

            def stage1(blki):
                if blki == 0:
                    xbt = xb0
                elif blki == 1:
                    xbt = xb1t
                else:
                    xbt = xpool.tile([P, KD, FD], BF16)
                    nc.sync.dma_start(xbt[:], xb[blki, :, :, :])
                hT = hpool.tile([P, MH, FD], BF16)
                xbs[blki] = hT
                m0 = 0
                if blki == 0:
                    # Split-k head: the x0 k4-7 chunk is a LATE head
                    # load, so run m0-3 over k0-3 first (start, no stop,
                    # one PSUM bank each) while it streams in, then
                    # finish those groups over k4-7. This fills the PE
                    # where it otherwise idles waiting for x.
                    psA = []
                    for m in range(4):
                        ps = ps1.tile([P, FD], F32)
                        psA.append(ps)
                        for k in range(4):
                            nc.tensor.matmul(
                                ps[:],
                                w1_sb[:, m, k * P:(k + 1) * P],
                                xbt[:, k, :],
                                start=(k == 0),
                                stop=False,
                            )
                    for m in range(4):
                        ps = psA[m]
                        for k in range(4, KD):
                            nc.tensor.matmul(
                                ps[:],
                                w1_sb[:, m, k * P:(k + 1) * P],
                                xbt[:, k, :],
                                start=False,
                                stop=(k == KD - 1),
                            )
                        relu_out(hT, m, ps)
                    m0 = 4
                for m in range(m0, MH):
                    ps = ps1.tile([P, FD], F32)
                    for k in range(KD):
                        nc.tensor.matmul(
                            ps[:],
                            w1_sb[:, m, k * P:(k + 1) * P],
                            xbt[:, k, :],
                            start=(k == 0),
                            stop=(k == KD - 1),
                        )
                    relu_out(hT, m, ps)

            def stage2(blki):
                last = blki == NB - 1
                hT = xbs.pop(blki)
                yt = ypool.tile([P, ND, FD], BF16)
                for d in range(ND):
                    # Final d-tile of the final block: split 384/128 so
                    # the very last copy + DMA after the last matmul
                    # cover only 128 columns (the DMA trigger cost is
                    # fixed at 128 descriptors; the copy and data scale).
                    halves = (
                        [(0, 384), (384, FD)]
                        if last and d == ND - 1 else [(0, FD)]
                    )
                    for c0, c1 in halves:
                        ps = ps2.tile([P, c1 - c0], F32)
                        for m in range(MH):
                            nc.tensor.matmul(
                                ps[:],
                                w2_sb[:, d, m * P:(m + 1) * P],
                                hT[:, m, c0:c1],
                                start=(m == 0),
                                stop=(m == MH - 1),
                            )
                        nc.vector.tensor_copy(yt[:, d, c0:c1], ps[:])
                        if last:  # stream the tail out: final DMA is tiny
                            nc.sync.dma_start(
                                yb[blki, :, d, c0:c1], yt[:, d, c0:c1]
                            )
                if not last:
                    nc.sync.dma_start(yb[blki, :, :, :], yt[:])

            # software-pipelined: stage 1 of block b+1 runs (on PE) between
            # stage 1 and stage 2 of block b, hiding the relu-eviction tail
            stage1(0)
            for b in range(NB):
                if b + 1 < NB:
                    stage1(b + 1)
                stage2(b)

    nc.compile()
    return nc


def _route(x_flat, router_w, router_b):
    """Replicates the reference router in numpy float32."""
    logits = x_flat @ router_w + router_b            # [N, E]
    m = logits.max(axis=-1, keepdims=True)
    p = np.exp(logits - m, dtype=np.float32)
    p /= p.sum(axis=-1, keepdims=True)
    # top-k, ties -> lower index first (matches jax.lax.top_k)
    top_i = np.argsort(-p, axis=-1, kind="stable")[:, :TOPK]
    top_p = np.take_along_axis(p, top_i, axis=-1)
    top_p = top_p / top_p.sum(axis=-1, keepdims=True)
    return top_p, top_i


def kernel(x, router_w, router_b, w1, b1, w2, b2, _trace=False):
    global last_run_results
    x = np.asarray(x, dtype=np.float32)
    router_w = np.asarray(router_w, dtype=np.float32)
    router_b = np.asarray(router_b, dtype=np.float32)
    w1 = np.asarray(w1, dtype=np.float32)
    b1 = np.asarray(b1, dtype=np.float32)
    w2 = np.asarray(w2, dtype=np.float32)
    b2 = np.asarray(b2, dtype=np.float32)

    N = B * S
    x_flat = x.reshape(N, D)
    top_p, top_i = _route(x_flat, router_w, router_b)

    # Tokens per expert (the "all-to-all by expert assignment")
    idx = [np.nonzero((top_i == e).any(axis=-1))[0] for e in range(E)]
    gates = [
        (top_p[idx[e]] * (top_i[idx[e]] == e)).sum(axis=-1) for e in range(E)
    ]
    counts = np.array([len(i) for i in idx])
    # Device capacity C=4096: the perfect-balance point (N*K/8). The few
    # overflow tokens of hotter-than-average experts are handled on the
    # host during the scatter-add (a data-parallel remainder).

    with_b1 = bool(np.any(b1))
    if with_b1 not in _cache:
        _cache[with_b1] = _build(with_b1=with_b1)
    nc = _cache[with_b1]

    in_maps = []
    for e in range(E):
        n_e = min(int(counts[e]), C)
        xTe = np.zeros((D, C), dtype=np.float32)
        xTe[:, :n_e] = x_flat[idx[e][:n_e]].T
        # [D, C] -> [NB, P, KD, FD] with D=(k p), C=(b c)
        xbe = np.ascontiguousarray(
            xTe.reshape(KD, P, NB, FD).transpose(2, 1, 0, 3)
        ).astype(ml_dtypes.bfloat16)
        w1pe = np.ascontiguousarray(
            w1[e].reshape(KD, P, MH, P).transpose(1, 2, 0, 3).reshape(P, MH, KD * P)
        ).astype(ml_dtypes.bfloat16)
        w2pe = np.ascontiguousarray(
            w2[e].reshape(MH, P, ND, P).transpose(1, 2, 0, 3).reshape(P, ND, MH * P)
        ).astype(ml_dtypes.bfloat16)
        im = {"xb": xbe, "w1p": w1pe, "w2p": w2pe}
        if with_b1:
            im["b1t"] = np.ascontiguousarray(b1[e].reshape(MH, P).T)
        in_maps.append(im)

    res = None
    for attempt in range(3):
        try:
            res = bass_utils.run_bass_kernel_spmd(
                nc, in_maps, list(range(N_CORES)), trace=_trace
            )
            break
        except Exception:
            if attempt == 2:
                raise
    last_run_results = res

    out_flat = np.zeros((N, D), dtype=np.float32)
    for e in range(E):
        n_e = min(int(counts[e]), C)
        ybe = np.asarray(res.results[e]["yb"]).astype(np.float32)
        # [NB, P, ND, FD] -> [C, D] with C=(b c), D=(d p)
        y_e = ybe.transpose(0, 3, 2, 1).reshape(C, D)[:n_e]
        out_flat[idx[e][:n_e]] += gates[e][:n_e, None] * (y_e + b2[e])
        if counts[e] > C:  # host handles the overflow tokens
            hi = idx[e][C:]
            h = np.maximum(x_flat[hi] @ w1[e] + b1[e], 0.0)
            y = h @ w2[e] + b2[e]
            out_flat[hi] += gates[e][C:, None] * y
    return out_flat.reshape(B, S, D)

